# revision 7
# baseline (speedup 1.0000x reference)
"""Trainium2 Bass kernel for the 4-layer sum/product circuit
(nn_KnowledgeLayer): h = enc(x); h = h[idx0].prod(1); h = h[idx1].sum(1);
h = h[idx2].prod(1); h = h[idx3].sum(1).

Strategy v7 (shard the COMPOSED SLOT STREAM, not the batch):
  * Host composes the four index maps into TWO flat operand streams of
    32768 row-indices each into a 4098-row full-batch enc table
    ([x | 1-x | 0 | 1], built host-side as [4098, 1024] fp16).
  * Core c owns h3 rows [c*512, (c+1)*512) and gathers FULL 2KB rows
    (all 1024 batch cols) for its slot subtree: 8192 gathers/core.
  * Two dma_gather (SWDGE) calls per 512-slot chunk (A and B streams)
    round-robin over 4 queues; DVE reduces fp16 mul/add/mul within
    partitions; the final sum pairs adjacent PARTITIONS via a PE
    matmul with a [128,64] pairing matrix (f32 PSUM); ACT drains
    PSUM; DMA writes 64 f32 output rows per chunk.

The bass program is identical for all 8 cores (pure SPMD); per-core
index streams differ via in_maps.
"""

import numpy as np

N_VARS = 2048
BATCH = 1024
NCORES = 8
TABLE = 2 * N_VARS + 2            # 4098
NOUT = 4096                       # h3 rows total
CORE_OUT = NOUT // NCORES         # 512 h3 rows per core
NCHUNK = 8
CHO = CORE_OUT // NCHUNK          # 64 h3 rows per chunk
CHS = CHO * 8                     # 512 h0 slots per chunk
IDXCOLS = 2 * NCHUNK * CHS // 16  # idx cols in meta (A|B interleaved)
MCOLS = IDXCOLS + CHO             # + pairs (f16 bits as i16)


# ----------------------------------------------------------------------------
# host-side index preparation
# ----------------------------------------------------------------------------

def _remap(e):
    """reference enc row -> our table row.
    table: [0,2048) = x[f], [2048,4096) = 1-x[f], 4096 = 0, 4097 = 1."""
    out = np.empty_like(e)
    out[e == 0] = 2 * N_VARS
    out[e == 1] = 2 * N_VARS + 1
    even = (e >= 2) & (e % 2 == 0)
    out[even] = (e[even] - 2) // 2
    odd = (e >= 3) & (e % 2 == 1)
    out[odd] = N_VARS + (e[odd] - 3) // 2
    return out


def _compose_indices(idx0, idx1, idx2, idx3):
    J = idx3.reshape(-1)              # [8192]  (i, a)   layer3 sum pairs
    K = idx2[J].reshape(-1)           # [16384] (i, a, b) layer2 prod pairs
    L = idx1[K].reshape(-1)           # [32768] (i, a, b, c) layer1 sum pairs
    AB = idx0[L]                      # [32768, 2]       layer0 prod pairs
    A = _remap(AB[:, 0].astype(np.int64))
    B = _remap(AB[:, 1].astype(np.int64))
    return A.reshape(NOUT, 2, 2, 2), B.reshape(NOUT, 2, 2, 2)


def _core_calls(S, c):
    """Per-core per-chunk call streams [NCHUNK, CHS].

    Chunk k covers i = c*512 + k*64 + ii.  Gather position within the
    stream: g = j*128 + p with free block j = cbit*2 + b and partition
    p = ii*2 + a, so h1 = h0[:, :2]+h0[:, 2:], h2 = h1[:, :1]*h1[:, 1:2],
    and the final a-sum pairs adjacent partitions (PE matmul).
    """
    Sc = S[c * CORE_OUT:(c + 1) * CORE_OUT]              # [512, 2, 2, 2]
    Sc = Sc.reshape(NCHUNK, CHO, 2, 2, 2)                # [k, ii, a, b, cb]
    Sc = Sc.transpose(0, 4, 3, 1, 2)                     # [k, cb, b, ii, a]
    return Sc.reshape(NCHUNK, CHS)


def _wrap(calls):
    """SWDGE wrap of per-chunk call streams: idx[p16, s] = call[s*16+p16]."""
    n, m = calls.shape
    w = calls.reshape(n, m // 16, 16).transpose(2, 0, 1).astype(np.int16)
    return w.reshape(16, n * (m // 16))


def _core_meta(A, B, pairs16, c):
    """Combined per-core input: wrapped A idx, wrapped B idx, pairs bits."""
    wa = _wrap(_core_calls(A, c))                        # [16, k*32]
    wb = _wrap(_core_calls(B, c))
    w = np.concatenate([wa, wb], axis=1)
    w = np.tile(w, (8, 1))                               # [128, IDXCOLS]
    return np.ascontiguousarray(
        np.concatenate([w, np.tile(pairs16.view(np.int16), (1, 1))], axis=1))


# ----------------------------------------------------------------------------
# bass program (built once, cached)
# ----------------------------------------------------------------------------

_CACHED = {}


def _build_program():
    import concourse.bacc as bacc
    import concourse.mybir as mybir
    from concourse.tile import TileContext

    f32 = mybir.dt.float32
    f16 = mybir.dt.float16
    i16 = mybir.dt.int16

    nc = bacc.Bacc("TRN2", target_bir_lowering=False, debug=False,
                   num_swdge_queues=4)

    enc = nc.dram_tensor("enc", [TABLE, BATCH], f16, kind="ExternalInput")
    meta = nc.dram_tensor("meta", [128, MCOLS], i16, kind="ExternalInput")
    out = nc.dram_tensor("out", [CORE_OUT, BATCH], f32, kind="ExternalOutput")

    with TileContext(nc) as tc:
        with tc.tile_pool(name="setup", bufs=1) as sp, \
             tc.tile_pool(name="gathera", bufs=4) as gpa, \
             tc.tile_pool(name="gatherb", bufs=4) as gpb, \
             tc.tile_pool(name="mid", bufs=2) as mp, \
             tc.tile_pool(name="hpsum", bufs=2, space="PSUM") as pp, \
             tc.tile_pool(name="outp", bufs=2) as outp:

            mt = sp.tile([128, MCOLS], i16, tag="mt")
            nc.sync.dma_start(out=mt[:, :], in_=meta[:, :])
            pr = mt[:, IDXCOLS:].bitcast(f16)
            boff = NCHUNK * CHS // 16       # B idx column offset
            cnt = nc.gpsimd.to_reg(CHS)

            ccols = CHS // 16               # 32 idx columns per chunk
            for k in range(NCHUNK):
                ga = gpa.tile([128, 4, BATCH], f16, tag="ga")
                gb = gpb.tile([128, 4, BATCH], f16, tag="gb")
                nc.gpsimd.dma_gather(
                    out_ap=ga[:, :, :], in_ap=enc[:, :],
                    idxs_ap=mt[:, k * ccols:(k + 1) * ccols],
                    num_idxs=CHS, num_idxs_reg=cnt,
                    elem_size=BATCH, queue_num=(2 * k) % 4)
                nc.gpsimd.dma_gather(
                    out_ap=gb[:, :, :], in_ap=enc[:, :],
                    idxs_ap=mt[:, boff + k * ccols:boff + (k + 1) * ccols],
                    num_idxs=CHS, num_idxs_reg=cnt,
                    elem_size=BATCH, queue_num=(2 * k + 1) % 4)

                h0 = mp.tile([128, 4, BATCH], f16, tag="h0")
                nc.vector.tensor_mul(h0[:, :, :], ga[:, :, :], gb[:, :, :])
                h1 = mp.tile([128, 2, BATCH], f16, tag="h1")
                nc.vector.tensor_add(
                    h1[:, :, :], h0[:, 0:2, :], h0[:, 2:4, :])
                h2 = mp.tile([128, 1, BATCH], f16, tag="h2")
                nc.vector.tensor_mul(
                    h2[:, :, :], h1[:, 0:1, :], h1[:, 1:2, :])

                # final sum pairs adjacent partitions: [128, 1024] -> [64, 1024]
                ps = pp.tile([CHO, BATCH], f32, tag="ps")
                for half in range(2):
                    nc.tensor.matmul(
                        ps[:, half * 512:(half + 1) * 512],
                        lhsT=pr,
                        rhs=h2[:, 0, half * 512:(half + 1) * 512],
                        start=True, stop=True)
                ot = outp.tile([CHO, BATCH], f32, tag="ot")
                nc.scalar.copy(ot[:, :], ps[:, :])
                nc.sync.dma_start(
                    out=out[k * CHO:(k + 1) * CHO, :], in_=ot[:, :])

    nc.compile()
    return nc


def _get_program():
    if "nc" not in _CACHED:
        _CACHED["nc"] = _build_program()
    return _CACHED["nc"]


# ----------------------------------------------------------------------------
# public entry point
# ----------------------------------------------------------------------------

def kernel(x, idx0, idx1, idx2, idx3, _trace=False, _trace_kwargs=None):
    from concourse.bass_utils import run_bass_kernel_spmd

    x = np.ascontiguousarray(np.asarray(x, dtype=np.float32))
    A, B = _compose_indices(
        np.asarray(idx0), np.asarray(idx1), np.asarray(idx2), np.asarray(idx3))

    enc = np.concatenate(
        [x, 1.0 - x,
         np.zeros((1, BATCH), np.float32),
         np.ones((1, BATCH), np.float32)], axis=0)
    enc = np.ascontiguousarray(enc.astype(np.float16))

    pairs16 = np.zeros((128, CHO), np.float16)
    pairs16[np.arange(128), np.arange(128) // 2] = 1.0

    nc = _get_program()
    in_maps = [{"enc": enc, "meta": _core_meta(A, B, pairs16, c)}
               for c in range(NCORES)]

    kwargs = {}
    if _trace:
        kwargs["trace"] = True
        if _trace_kwargs:
            kwargs.update(_trace_kwargs)
    res = run_bass_kernel_spmd(nc, in_maps, core_ids=list(range(NCORES)), **kwargs)
    outs = [res.results[c]["out"] for c in range(NCORES)]
    full = np.concatenate(outs, axis=0)
    if _trace:
        kernel.last_exec_time_ns = res.exec_time_ns
        kernel.last_profile = res.profile_json
    return full


# revision 9
# speedup vs baseline: 1.1750x; 1.1750x over previous
"""Trainium2 Bass kernel for the 4-layer sum/product circuit
(nn_KnowledgeLayer): h = enc(x); h = h[idx0].prod(1); h = h[idx1].sum(1);
h = h[idx2].prod(1); h = h[idx3].sum(1).

Strategy v8 (shard the COMPOSED SLOT STREAM, not the batch):
  * Host composes the four index maps into TWO flat operand streams of
    32768 row-indices each into a 4098-row full-batch enc table
    ([x | 1-x | 0 | 1], built host-side as [4098, 1024] fp16).
  * Core c owns h3 rows [c*512, (c+1)*512) and gathers FULL 2KB rows
    (all 1024 batch cols) for its slot subtree: 8192 gathers/core.
  * Index loads are split across the three HWDGE engines (Sync/Scalar/
    Vector) with a small chunk-0 slice first so the first dma_gather
    fires ~8us in instead of ~18us.
  * Two dma_gather (SWDGE) calls per 512-slot chunk round-robin over
    4 queues; DVE reduces fp16 mul/add/mul within partitions; the
    final sum pairs adjacent PARTITIONS via a PE matmul with a
    [128,64] pairing matrix (f32 PSUM); ACT drains PSUM; DMA writes
    64 f32 output rows per chunk.

The bass program is identical for all 8 cores (pure SPMD); per-core
index streams differ via in_maps.
"""

import numpy as np

N_VARS = 2048
BATCH = 1024
NCORES = 8
TABLE = 2 * N_VARS + 2            # 4098
NOUT = 4096                       # h3 rows total
CORE_OUT = NOUT // NCORES         # 512 h3 rows per core
NCHUNK = 8
CHO = CORE_OUT // NCHUNK          # 64 h3 rows per chunk
CHS = CHO * 8                     # 512 h0 slots per chunk


# ----------------------------------------------------------------------------
# host-side index preparation
# ----------------------------------------------------------------------------

def _remap(e):
    """reference enc row -> our table row.
    table: [0,2048) = x[f], [2048,4096) = 1-x[f], 4096 = 0, 4097 = 1."""
    out = np.empty_like(e)
    out[e == 0] = 2 * N_VARS
    out[e == 1] = 2 * N_VARS + 1
    even = (e >= 2) & (e % 2 == 0)
    out[even] = (e[even] - 2) // 2
    odd = (e >= 3) & (e % 2 == 1)
    out[odd] = N_VARS + (e[odd] - 3) // 2
    return out


def _compose_indices(idx0, idx1, idx2, idx3):
    J = idx3.reshape(-1)              # [8192]  (i, a)   layer3 sum pairs
    K = idx2[J].reshape(-1)           # [16384] (i, a, b) layer2 prod pairs
    L = idx1[K].reshape(-1)           # [32768] (i, a, b, c) layer1 sum pairs
    AB = idx0[L]                      # [32768, 2]       layer0 prod pairs
    A = _remap(AB[:, 0].astype(np.int64))
    B = _remap(AB[:, 1].astype(np.int64))
    return A.reshape(NOUT, 2, 2, 2), B.reshape(NOUT, 2, 2, 2)


def _core_wrap(S, c):
    """Per-core chunked+wrapped int16 index tensor [128, NCHUNK*CHS//16].

    Chunk k covers i = c*512 + k*64 + ii.  Gather position within a call:
    g = j*128 + p with free block j = cbit*2 + b and partition p = ii*2+a,
    so h1 = h0[:, :2]+h0[:, 2:], h2 = h1[:, :1]*h1[:, 1:2], and the final
    a-sum pairs adjacent partitions (PE matmul).
    SWDGE wraps each call's g-stream: idx[p16, s] = call[s*16 + p16].
    """
    Sc = S[c * CORE_OUT:(c + 1) * CORE_OUT]              # [512, 2, 2, 2]
    Sc = Sc.reshape(NCHUNK, CHO, 2, 2, 2)                # [k, ii, a, b, cb]
    Sc = Sc.transpose(0, 4, 3, 1, 2)                     # [k, cb, b, ii, a]
    calls = Sc.reshape(NCHUNK, CHS)                      # g = ((cb*2+b)*64+ii)*2+a
    w = calls.reshape(NCHUNK, CHS // 16, 16)             # [k, s, p16]
    w = w.transpose(2, 0, 1).astype(np.int16)            # [16, k, s]
    w = w.reshape(16, NCHUNK * (CHS // 16))
    return np.ascontiguousarray(np.tile(w, (8, 1)))      # [128, k*32]


# ----------------------------------------------------------------------------
# bass program (built once, cached)
# ----------------------------------------------------------------------------

_CACHED = {}


def _build_program():
    import concourse.bacc as bacc
    import concourse.mybir as mybir
    from concourse.tile import TileContext

    f32 = mybir.dt.float32
    f16 = mybir.dt.float16
    i16 = mybir.dt.int16

    nc = bacc.Bacc("TRN2", target_bir_lowering=False, debug=False,
                   num_swdge_queues=4)

    enc = nc.dram_tensor("enc", [TABLE, BATCH], f16, kind="ExternalInput")
    idxa = nc.dram_tensor("idxa", [128, NCHUNK * CHS // 16], i16,
                          kind="ExternalInput")
    idxb = nc.dram_tensor("idxb", [128, NCHUNK * CHS // 16], i16,
                          kind="ExternalInput")
    pairs = nc.dram_tensor("pairs", [128, CHO], f16, kind="ExternalInput")
    out = nc.dram_tensor("out", [CORE_OUT, BATCH], f32, kind="ExternalOutput")

    with TileContext(nc) as tc:
        with tc.tile_pool(name="setup", bufs=1) as sp, \
             tc.tile_pool(name="gather", bufs=4) as gp, \
             tc.tile_pool(name="mid", bufs=2) as mp, \
             tc.tile_pool(name="hpsum", bufs=2, space="PSUM") as pp, \
             tc.tile_pool(name="outp", bufs=2) as outp:

            ia = sp.tile([128, NCHUNK * CHS // 16], i16, tag="ia")
            ib = sp.tile([128, NCHUNK * CHS // 16], i16, tag="ib")
            pr = sp.tile([128, CHO], f16, tag="pr")
            ccols = CHS // 16        # 32 idx columns per chunk
            # chunk-0 slices first on two engines, the rest behind, so the
            # first gather is not gated on the full 64KB idx transfers
            nc.sync.dma_start(out=ia[:, :ccols], in_=idxa[:, :ccols])
            nc.scalar.dma_start(out=ib[:, :ccols], in_=idxb[:, :ccols])
            nc.sync.dma_start(out=ia[:, ccols:], in_=idxa[:, ccols:])
            nc.scalar.dma_start(out=ib[:, ccols:], in_=idxb[:, ccols:])
            nc.scalar.dma_start(out=pr[:, :], in_=pairs[:, :])
            cnt = nc.gpsimd.to_reg(CHS)

            for k in range(NCHUNK):
                ga = gp.tile([128, 4, BATCH], f16, tag="ga")
                gb = gp.tile([128, 4, BATCH], f16, tag="gb")
                nc.gpsimd.dma_gather(
                    out_ap=ga[:, :, :], in_ap=enc[:, :],
                    idxs_ap=ia[:, k * ccols:(k + 1) * ccols],
                    num_idxs=CHS, num_idxs_reg=cnt,
                    elem_size=BATCH, queue_num=(2 * k) % 4)
                nc.gpsimd.dma_gather(
                    out_ap=gb[:, :, :], in_ap=enc[:, :],
                    idxs_ap=ib[:, k * ccols:(k + 1) * ccols],
                    num_idxs=CHS, num_idxs_reg=cnt,
                    elem_size=BATCH, queue_num=(2 * k + 1) % 4)

                h0 = mp.tile([128, 4, BATCH], f16, tag="h0")
                nc.vector.tensor_mul(h0[:, :, :], ga[:, :, :], gb[:, :, :])
                h1 = mp.tile([128, 2, BATCH], f16, tag="h1")
                nc.vector.tensor_add(
                    h1[:, :, :], h0[:, 0:2, :], h0[:, 2:4, :])
                h2 = mp.tile([128, 1, BATCH], f16, tag="h2")
                nc.vector.tensor_mul(
                    h2[:, :, :], h1[:, 0:1, :], h1[:, 1:2, :])

                # final sum pairs adjacent partitions: [128, 1024] -> [64, 1024]
                ps = pp.tile([CHO, BATCH], f32, tag="ps")
                for half in range(2):
                    nc.tensor.matmul(
                        ps[:, half * 512:(half + 1) * 512],
                        lhsT=pr[:, :],
                        rhs=h2[:, 0, half * 512:(half + 1) * 512],
                        start=True, stop=True)
                ot = outp.tile([CHO, BATCH], f32, tag="ot")
                nc.scalar.copy(ot[:, :], ps[:, :])
                nc.sync.dma_start(
                    out=out[k * CHO:(k + 1) * CHO, :], in_=ot[:, :])

    nc.compile()
    return nc


def _get_program():
    if "nc" not in _CACHED:
        _CACHED["nc"] = _build_program()
    return _CACHED["nc"]


# ----------------------------------------------------------------------------
# public entry point
# ----------------------------------------------------------------------------

def kernel(x, idx0, idx1, idx2, idx3, _trace=False, _trace_kwargs=None):
    from concourse.bass_utils import run_bass_kernel_spmd

    x = np.ascontiguousarray(np.asarray(x, dtype=np.float32))
    A, B = _compose_indices(
        np.asarray(idx0), np.asarray(idx1), np.asarray(idx2), np.asarray(idx3))

    enc = np.concatenate(
        [x, 1.0 - x,
         np.zeros((1, BATCH), np.float32),
         np.ones((1, BATCH), np.float32)], axis=0)
    enc = np.ascontiguousarray(enc.astype(np.float16))

    pairs = np.zeros((128, CHO), np.float16)
    pairs[np.arange(128), np.arange(128) // 2] = 1.0

    nc = _get_program()
    in_maps = [{"enc": enc, "idxa": _core_wrap(A, c), "idxb": _core_wrap(B, c),
                "pairs": pairs}
               for c in range(NCORES)]

    kwargs = {}
    if _trace:
        kwargs["trace"] = True
        if _trace_kwargs:
            kwargs.update(_trace_kwargs)
    res = run_bass_kernel_spmd(nc, in_maps, core_ids=list(range(NCORES)), **kwargs)
    outs = [res.results[c]["out"] for c in range(NCORES)]
    full = np.concatenate(outs, axis=0)
    if _trace:
        kernel.last_exec_time_ns = res.exec_time_ns
        kernel.last_profile = res.profile_json
    return full
